# revision 10
# baseline (speedup 1.0000x reference)
# Contrastive loss (L2-distance scores, margin hinge, mean reduction) on 8
# Trainium2 NeuronCores.
#
# total = mean(cost_s) + mean(cost_im) over the [N, N] score matrix
#   scores[i, j] = -||im_i - s_j||
#   cost_s  = relu(margin + scores - diag_row)   (diag zeroed)
#   cost_im = relu(margin + scores - diag_col)   (diag zeroed)
#
# Identity used on device:  relu(a - d) = max(a, d) - d, so
#   sum(cost_s) + sum(cost_im) = S1 + S2 - 2*SD
#   S1 = sum_ij max(a_i, D_ij), S2 = sum_ij max(b_j, D_ij), SD = sum_ij D_ij
# with a_i = b_i = margin + ||im_i - s_i||.
#
# Sharding: rows of the score matrix across 8 cores ([1024, 8192] slab each).
# s (columns) is replicated but column-ROTATED per core so each core's
# diagonal block lands at local columns [0, 1024) -> one static SPMD program.
# The diagonal is zeroed exactly by subtracting BIG from the PSUM diagonal
# before the sqrt: then max(a, D_ii)=D_ii and the diag contributes 0 to
# S1 + S2 - 2*SD.
#
# Per (m-tile, group-of-2048-cols):
#   PE  : 2x K=128 bf16 matmuls (dot) + 1x K=2 ones-matmul adding a bf16
#         hi/lo split of -||s_j||^2/2  ->  PSUM q = dot - s_sq/2
#   DVE : (group 0 only) in-place add of -BIG*eye on the diagonal block
#   ACT : D = sqrt(-2*q + im_sq_i)  [PSUM->SBUF bf16], accum_out -> SD
#   DVE : tensor_scalar  max(D, a_i)        accum_out -> S1   (4x mode)
#   DVE : scalar_tensor_tensor max(D, b_j)  accum_out -> S2   (2x mode)
# Final: reduce accum columns, partition-sum via ones-matmul, DMA scalar out.
# Host: sum 8 partials, divide by N^2.

import numpy as np
import ml_dtypes

import concourse.bass as bass
import concourse.tile as tile
from concourse import bacc, mybir
from concourse import bass_utils
from concourse import dve_ops as _dve_ops
from concourse.dve_spec import Spec as _DveSpec, Src0, Src1, C0, C1, lower as _dve_lower, maxx as _dve_maxx
from concourse.dve_uop import DveOpSpec as _DveOpSpec

N = 8192
D = 256
MARGIN = 0.2
NCORES = 8
SLAB = N // NCORES          # 1024 rows per core
MT = SLAB // 128            # 8 m-tiles per core
GROUP = 2048                # columns per PSUM group (4 banks)
NG = N // GROUP             # 4 groups
CHUNK = 512                 # columns per matmul (1 PSUM bank)
NCHUNK = GROUP // CHUNK     # 4 chunks per group
BIG = 5.0e4

BF16 = ml_dtypes.bfloat16
_F = mybir.dt.float32
_B = mybir.dt.bfloat16
_R = mybir.dt.float32r


def _register_hinge_op():
    """Register a fused custom DVE op computing the whole per-element hinge:

        out = max(Src0, C0) + max(Src0, Src1) - Src0 * C1
        accum_out = sum(out)

    with Src0 = D (distance tile), Src1 = b broadcast row, C0 = a per-partition
    scalar, C1 = 2.0. One 1x DVE pass replaces TS-max(4x) + STT-max(1x) +
    the separate sum-of-D accumulation.
    """
    if any(op.name == "CONTRASTIVE_HINGE_ANT" for op in _dve_ops.OPS):
        return next(op for op in _dve_ops.OPS if op.name == "CONTRASTIVE_HINGE_ANT")

    def _ref(in0, in1, s0, s1, imm2):
        x = in0.astype(np.float32)
        body = (
            np.maximum(x, s0) + np.maximum(x, in1.astype(np.float32)) - x * s1
        ).astype(np.float32)
        return body, body.reshape(body.shape[0], -1).sum(axis=-1, keepdims=True)

    from operator import add as _add

    spec = _DveSpec(
        body=_dve_maxx(Src0, C0) + _dve_maxx(Src0, Src1) - Src0 * C1,
        accum=_add,
        reference=_ref,
    )
    # sha is pinned at runtime from our own lowering (no source tree to edit).
    shas = {}
    for ver in ("v3", "v4"):
        try:
            s = _DveOpSpec(
                name="CONTRASTIVE_HINGE_ANT",
                opcode=0,
                uops=_dve_lower(spec, ver=ver),
                rd1_en=True,
            )
            shas[ver] = s.sha(ver)
        except Exception:
            pass
    op = _dve_ops.DveOp(
        "CONTRASTIVE_HINGE_ANT", spec, subdim=False, uops_sha=shas
    )
    _dve_ops.OPS.append(op)
    _dve_ops._SUB_OPCODE_FOR_NAME[op.name] = (
        _dve_ops._CUSTOM_DVE_ROW_BASE + len(_dve_ops.OPS) - 1
    )
    return op


def build_module():
    """Trace + compile the per-core Bass module (one SPMD NEFF for 8 cores)."""
    nc = bacc.Bacc("TRN2", num_devices=NCORES)

    imT = nc.dram_tensor("imT", [2, 128, SLAB], _R, kind="ExternalInput")
    sT = nc.dram_tensor("sT", [2, 128, N], _R, kind="ExternalInput")
    fold = nc.dram_tensor("fold", [2, N], _B, kind="ExternalInput")
    brow = nc.dram_tensor("brow", [N], _F, kind="ExternalInput")
    avec = nc.dram_tensor("avec", [128, MT], _F, kind="ExternalInput")
    imsq = nc.dram_tensor("imsq", [128, MT], _F, kind="ExternalInput")
    eyeb = nc.dram_tensor("eyeb", [128, 128], _F, kind="ExternalInput")
    out = nc.dram_tensor("out", [1, 1], _F, kind="ExternalOutput")

    with tile.TileContext(nc) as tc:
        with (
            tc.tile_pool(name="singles", bufs=1) as singles,
            tc.tile_pool(name="dtiles", bufs=3) as dpool,
            tc.tile_pool(name="trash", bufs=2) as tpool,
            tc.tile_pool(name="psum", bufs=2, space="PSUM") as ppool,
        ):
            # ---- resident inputs (DMAs spread across engine queues) --------------
            dma_engines = [nc.sync, nc.scalar, nc.gpsimd]
            _dmai = [0]

            def dma(out_ap, in_ap):
                eng = dma_engines[_dmai[0] % len(dma_engines)]
                _dmai[0] += 1
                eng.dma_start(out=out_ap, in_=in_ap)

            lhs_sb = singles.tile([128, 2, SLAB], _R)
            for k in range(2):
                dma(lhs_sb[:, k, :], imT.ap()[k])
            rhs_sb = singles.tile([128, 2, N], _R)
            for g in range(NG):
                for k in range(2):
                    cols = slice(g * GROUP, (g + 1) * GROUP)
                    dma(rhs_sb[:, k, cols], sT.ap()[k, :, cols])
            # fold rows replicated at 4 base partitions for row-group packing
            fold_sb = singles.tile([128, N], _B)
            for bp in (0, 32, 64, 96):
                dma(fold_sb[bp : bp + 2, :], fold.ap())
            b_sb = singles.tile([128, N], _F)
            brow_bcast = bass.AP(
                tensor=brow.ap().tensor, offset=0, ap=[[0, 128], [1, N]]
            )
            dma(b_sb[:], brow_bcast)
            avec_sb = singles.tile([128, MT], _F)
            dma(avec_sb[:], avec.ap())
            imsq_sb = singles.tile([128, MT], _F)
            dma(imsq_sb[:], imsq.ap())
            eyeb_sb = singles.tile([128, 128], _F)
            dma(eyeb_sb[:], eyeb.ap())

            ones2 = singles.tile([128, 128], _B)
            nc.vector.memset(ones2[:], 1.0)
            ones_col = singles.tile([128, 1], _F)
            nc.vector.memset(ones_col[:], 1.0)

            acc = singles.tile([128, MT * NG], _F)
            hinge_op = _register_hinge_op()

            # ---- main loop -------------------------------------------------------
            for m in range(MT):
                lhs0 = lhs_sb[:, 0, m * 128 : (m + 1) * 128]
                lhs1 = lhs_sb[:, 1, m * 128 : (m + 1) * 128]
                a_col = avec_sb[:, m : m + 1]
                q_col = imsq_sb[:, m : m + 1]
                for g in range(NG):
                    ps = ppool.tile([128, GROUP], _F, tag="psum")
                    for c in range(NCHUNK):
                        pslice = ps[:, c * CHUNK : (c + 1) * CHUNK]
                        cols = slice(g * GROUP + c * CHUNK, g * GROUP + (c + 1) * CHUNK)
                        nc.tensor.matmul(
                            pslice, lhsT=lhs0, rhs=rhs_sb[:, 0, cols],
                            start=True, stop=False,
                        )
                        nc.tensor.matmul(
                            pslice, lhsT=lhs1, rhs=rhs_sb[:, 1, cols],
                            start=False, stop=False,
                        )
                    # K=2 fold matmuls, packed 4-wide into disjoint row groups
                    # (0/32/64/96) so they run concurrently on the PE array.
                    for c in range(NCHUNK):
                        bp = 32 * c
                        pslice = ps[:, c * CHUNK : (c + 1) * CHUNK]
                        cols = slice(g * GROUP + c * CHUNK, g * GROUP + (c + 1) * CHUNK)
                        nc.tensor.matmul(
                            pslice,
                            lhsT=ones2[bp : bp + 2, :],
                            rhs=fold_sb[bp : bp + 2, cols],
                            start=False, stop=True,
                            tile_position=(bp, 0),
                        )
                    if g == 0:
                        # diagonal block of this m-tile: local cols [128m, 128m+128)
                        dslice = ps[:, m * 128 : (m + 1) * 128]
                        nc.vector.tensor_tensor(
                            out=dslice, in0=dslice, in1=eyeb_sb[:],
                            op=mybir.AluOpType.add,
                        )
                    col = m * NG + g
                    dt = dpool.tile([128, GROUP], _F, tag="dt")
                    nc.scalar.activation(
                        out=dt[:], in_=ps[:],
                        func=mybir.ActivationFunctionType.Sqrt,
                        bias=q_col, scale=-2.0,
                    )
                    t1 = tpool.tile([128, GROUP], _F, tag="t1")
                    nc.vector._custom_dve(
                        hinge_op,
                        out=t1[:],
                        in0=dt[:],
                        in1=b_sb[:, g * GROUP : (g + 1) * GROUP],
                        s0=a_col,
                        s1=2.0,
                        accum_out=acc[:, col : col + 1],
                    )

            # ---- combine ---------------------------------------------------------
            total_col = singles.tile([128, 1], _F)
            nc.vector.tensor_reduce(
                out=total_col[:], in_=acc[:], axis=mybir.AxisListType.X,
                op=mybir.AluOpType.add,
            )

            fps = ppool.tile([1, 1], _F, tag="psum")
            nc.tensor.matmul(fps[:], lhsT=total_col[:], rhs=ones_col[:],
                             start=True, stop=True)
            out_sb = singles.tile([1, 1], _F)
            nc.vector.tensor_copy(out=out_sb[:], in_=fps[:])
            nc.sync.dma_start(out=out.ap(), in_=out_sb[:])

    nc.compile()
    return nc


def prepare_inputs(im: np.ndarray, s: np.ndarray):
    """Host-side sharding + dtype conversion. Returns in_maps for 8 cores."""
    im = np.ascontiguousarray(im, dtype=np.float32)
    s = np.ascontiguousarray(s, dtype=np.float32)

    im64 = im.astype(np.float64)
    s64 = s.astype(np.float64)
    diag_true = np.sqrt(((im64 - s64) ** 2).sum(1))          # [N] exact
    b_full = (MARGIN + diag_true).astype(np.float32)         # [N]

    im_sq = (im64**2).sum(1).astype(np.float32)              # [N]
    s_sq = (s64**2).sum(1)                                   # [N] f64
    foldv = -0.5 * s_sq
    fold_hi = foldv.astype(np.float32).astype(BF16)
    fold_lo = (foldv - fold_hi.astype(np.float64)).astype(np.float32).astype(BF16)

    eyeb = (np.eye(128, dtype=np.float32) * np.float32(-BIG))

    in_maps = []
    for c in range(NCORES):
        rows = slice(c * SLAB, (c + 1) * SLAB)
        rot = np.roll(np.arange(N), -c * SLAB)
        imT = np.ascontiguousarray(im[rows].T.reshape(2, 128, SLAB))
        sT = np.ascontiguousarray(s[rot].T.reshape(2, 128, N))
        foldc = np.ascontiguousarray(np.stack([fold_hi[rot], fold_lo[rot]]))
        browc = np.ascontiguousarray(b_full[rot])
        avecc = np.ascontiguousarray(b_full[rows].reshape(MT, 128).T)
        imsqc = np.ascontiguousarray(im_sq[rows].reshape(MT, 128).T)
        in_maps.append(
            {
                "imT": imT,
                "sT": sT,
                "fold": foldc,
                "brow": browc,
                "avec": avecc,
                "imsq": imsqc,
                "eyeb": eyeb,
            }
        )
    return in_maps


_NC_CACHE = None


def get_module():
    global _NC_CACHE
    if _NC_CACHE is None:
        _NC_CACHE = build_module()
    return _NC_CACHE


def kernel(im: np.ndarray, s: np.ndarray) -> np.ndarray:
    nc = get_module()
    in_maps = prepare_inputs(im, s)
    res = bass_utils.run_bass_kernel_spmd(
        nc, in_maps, core_ids=list(range(NCORES))
    )
    total = 0.0
    for c in range(NCORES):
        total += float(res.results[c]["out"][0, 0])
    return np.array(np.float64(total) / (N * N), dtype=np.float32)
